# revision 1
# baseline (speedup 1.0000x reference)
"""CRF Viterbi decode kernel for Trainium2 (8 NeuronCores, pure data parallel).

Problem: X [4096, 512, 128] f32, W [26, 128], T [26, 26]
  e = einsum('bld,kd->blk', X, W)
  per word: Viterbi max-sum scan over L=512 with transition T, backtrace,
  output int32 labels [4096, 512].

Strategy (per core, 512 words):
  - shard batch across 8 cores (512 words each), replicate W/T.
  - words on partitions: 4 groups of 128 words.
  - emissions on PE: per (group, position) transpose X tile [128w,128d] ->
    [128d,128w] (PE transpose), matmul with W^T -> e [128w, 26] in PSUM,
    batch-copy to SBUF, spill e to DRAM (needed again by backward scan).
  - forward scan (l) and backward scan (G) instead of backtrace:
      l[i,y] = max_j(e[i-1,j] + T[j,y] + l[i-1,j])     (reference's scan)
      G[i,j] = e[i,j] + max_y(T[j,y] + G[i+1,y]),  G[L-1] = e[L-1]
      label[i] = argmax_y(l[i,y] + G[i,y])
    We keep m = e + l (the quantity the scan consumes anyway); then
    t[i] = m[i] + maxpart[i] where G[i] = e[i] + maxpart[i].
  - the inner max-plus step uses a hand-built custom DVE op SEG_MAX_ADD_ANT:
    out[p,s,k] = running max over k within page s of (in0+in1), so
    out[:, y, 25] = max_j(m[j] + T[j,y]) in ONE pass (fused add+max with
    per-page reset via the SUB_DIM_DONE step-state, like TENSOR_PAGED_MASK).
  - m is spilled to DRAM during the forward pass, streamed back (with e)
    during the backward pass; labels via bulk masked argmax per chunk.
"""

import os
import sys

for _p in ("/opt/trn_rl_repo", "/root/.axon_site/_ro/trn_rl_repo"):
    if os.path.isdir(_p) and _p not in sys.path:
        sys.path.append(_p)

import dataclasses
import numpy as np

import concourse.bass as bass
import concourse.tile as tile
from concourse import bacc, mybir
from concourse.bass_utils import run_bass_kernel_spmd

F32 = mybir.dt.float32
I32 = mybir.dt.int32

NUM_CORES = 8
K = 26
D = 128
NG = 4  # word groups of 128 per core

# ---------------------------------------------------------------------------
# Custom DVE op: segmented (per-page) running max of (Src0 + Src1).
# ---------------------------------------------------------------------------

_SEGMAX_NAME = "SEG_MAX_ADD_ANT"
_segmax_op = None


def _segmax_reference(in0, in1, c0, c1, c2):
    z = np.asarray(in0, np.float32) + np.asarray(in1, np.float32)
    return np.maximum.accumulate(z, axis=-1)


def _build_segmax_uops(ver):
    from concourse import dve_spec as ds
    from concourse.dve_spec import AluOp, Spec, Src0, Src1, Trigger, scan

    spec = Spec(body=scan(AluOp.MAX, Src0 + Src1), reference=_segmax_reference)
    ds._validate_body(spec, ver)
    spec2 = ds._hoist_stream_invariant_ops(spec)
    scans = ds._collect(spec2.body, ds.Scan)
    placement = ds._build_placement(spec2, scans, ds.N_STAGES[ver], ds.N_LANES[ver])
    base = ds._build_state_machine(spec2, scans, [], placement)
    assert len(base) == 2  # [seed, steady]
    d = placement.node_stage[scans[0]]
    steady_stage = placement.pipeline[d]
    seed = base[0]
    steady = dataclasses.replace(
        base[1],
        trigger=(Trigger.SRC_TENSOR_DONE, Trigger.SUB_DIM_DONE, Trigger.NONE),
        next=(0, 2, 0),
    )
    # Boundary element: recombine against -FLT_MAX instead of the running
    # value -> the fold restarts at each page, exactly the PageIdx step-state
    # shape with the combine kept.
    step = dataclasses.replace(
        base[1],
        trigger=(Trigger.SRC_TENSOR_DONE, Trigger.SUB_DIM_DONE, Trigger.COUNT),
        next=(0, 2, 1),
        repeat=1,
        overrides={d: ds._Stage(AluOp.MAX, ds.MaxNeg, steady_stage.b)},
    )
    uops = [ds._assemble(s) for s in (seed, steady, step)]
    for u in uops:
        u.validate(ver)
    return spec, uops


def get_segmax_op():
    """Build + register the custom op in the dve_ops registries (idempotent)."""
    global _segmax_op
    if _segmax_op is not None:
        return _segmax_op
    from concourse import dve_ops
    from concourse.dve_ops import OPS, CUSTOM_DVE_SPECS, _SUB_OPCODE_FOR_NAME, DveOp
    from concourse.dve_uop import DveOpSpec

    @dataclasses.dataclass(frozen=True)
    class _HandBuiltDveOp(DveOp):
        def compile(self, ver):
            key = (self.name, ver)
            if (r := dve_ops._COMPILE_CACHE.get(key)) is not None:
                return r
            from concourse.dve_ops import get_dve_sub_opcode

            _, uops = _build_segmax_uops(ver)
            result = DveOpSpec(
                name=self.name,
                opcode=get_dve_sub_opcode(self.name),
                uops=uops,
                rd1_en=True,
            )
            dve_ops._COMPILE_CACHE[key] = result
            return result

    spec, _ = _build_segmax_uops("v3")
    op = _HandBuiltDveOp(_SEGMAX_NAME, spec, subdim=True, uops_sha={})
    if _SEGMAX_NAME not in _SUB_OPCODE_FOR_NAME:
        OPS.append(op)
        CUSTOM_DVE_SPECS[_SEGMAX_NAME] = spec
        _SUB_OPCODE_FOR_NAME[_SEGMAX_NAME] = 1 + len(OPS) - 1
        assert _SUB_OPCODE_FOR_NAME[_SEGMAX_NAME] < 0x20
    _segmax_op = op
    return op


def _custom_dve_raw(vec, op, out, in0, in1):
    """_custom_dve minus the <=2-free-dim assert: emit one InstCustomDveAnt
    over 3-free-dim APs so all 4 word groups ride in a single instruction.
    SUB_DIM_DONE fires on every innermost-dim wrap, so per-(g,y)-page reset
    semantics are unchanged."""
    import concourse.bass_isa as bass_isa
    from concourse.bass import MemorySpace
    from concourse.dve_ops import get_dve_sub_opcode
    from concourse.dve_table_gen import dve_ver_for
    from concourse import mybir as mb

    nc = vec.bass
    if op.name not in nc.m.ant_custom_dve_ops:
        nc.m.ant_custom_dve_ops = sorted({*nc.m.ant_custom_dve_ops, op.name})
    compiled = op.compile(dve_ver_for(nc.trn_type))
    shape = bass_isa.CustomDveShape.STT  # in1 has 2+ free dims
    isa_opcode = nc.isa.Opcode[
        f"NEURON_ISA_TPB_OPCODE_CUSTOM_DVE_ANT_{shape.slot()}"
    ].value
    imm = mb.ImmediateValue(dtype=mb.dt.float32, value=0.0)
    return vec.add_instruction(
        bass_isa.InstCustomDveAnt(
            name=nc.get_next_instruction_name(),
            op_name=op.name,
            rd1_en=compiled.rd1_en,
            subdim=0x02 if op.subdim else 0,
            imm2=0.0,
            shape=shape,
            row=get_dve_sub_opcode(op.name),
            isa_opcode=isa_opcode,
            ins=[
                vec.lower_ap(in0, for_isa=True, opt=False),
                vec.lower_ap(in1, for_isa=True, opt=False),
                imm,
                mb.ImmediateValue(dtype=mb.dt.float32, value=0.0),
            ],
            outs=[vec.lower_ap(out, for_isa=True, opt=False)],
        )
    )


# ---------------------------------------------------------------------------
# Program builder
# ---------------------------------------------------------------------------


def build_crf_kernel(ctx, tc, out_aps, in_aps, L=512, use_fused=True, adds_engine="vector"):
    """Emit the per-core CRF decode program.

    in_aps: dict with DRAM APs: X [NG*128, L, D], wt [128, K] (= W^T),
      trepf [128, K*K] (T^T flat: [y*26+j] = T[j,y]),
      trepg [128, K*K] (T flat:  [j*26+y] = T[j,y]),
      ident [128, 128], revy [128, K] (= 26 - y).
    out_aps: dict with labels [NG*128, L] int32.
    """
    nc = tc.nc
    X = in_aps["X"]
    labels_out = out_aps["labels"]
    adds = nc.gpsimd if adds_engine == "gpsimd" else nc.vector

    KK = K * K
    CH = min(32, L)  # scan/spill chunk
    assert L % CH == 0
    NCH = L // CH
    XCH = min(8, L)  # X staging sub-chunk
    EPS = min(16, CH)  # emissions per PSUM bank tile (16*26=416 fp32 <= 512)

    segmax = get_segmax_op() if use_fused else None

    # DRAM scratch for e and m, layout [i, w, k] (i-major so a chunk of
    # positions for all words is one contiguous region).
    e_dram = nc.dram_tensor(f"e_scr_{L}", [L, NG * 128, K], F32).ap()
    m_dram = nc.dram_tensor(f"m_scr_{L}", [L, NG * 128, K], F32).ap()

    cpool = ctx.enter_context(tc.tile_pool(name="consts", bufs=1))
    trepf = cpool.tile([128, KK], F32, tag="trepf")
    trepg = cpool.tile([128, KK], F32, tag="trepg")
    wt = cpool.tile([128, K], F32, tag="wt")
    ident = cpool.tile([128, 128], F32, tag="ident")
    revy = cpool.tile([128, K], F32, tag="revy")
    nc.sync.dma_start(trepf[:], in_aps["trepf"])
    nc.sync.dma_start(trepg[:], in_aps["trepg"])
    nc.sync.dma_start(wt[:], in_aps["wt"])
    nc.sync.dma_start(ident[:], in_aps["ident"])
    nc.sync.dma_start(revy[:], in_aps["revy"])
    trepf3 = trepf[:].rearrange("p (y j) -> p y j", j=K)
    trepg3 = trepg[:].rearrange("p (j y) -> p j y", y=K)

    labels_pool = ctx.enter_context(tc.tile_pool(name="labels", bufs=1))
    labels_sb = labels_pool.tile([128, NG, L], I32)

    # ---------------- Phase A: emissions + forward scan ----------------
    with (
        tc.tile_pool(name="xstage", bufs=2) as pool_x,
        tc.tile_pool(name="esb", bufs=3) as pool_esb,
        tc.tile_pool(name="mout", bufs=3) as pool_m,
        tc.tile_pool(name="sall", bufs=3) as pool_s,
        tc.tile_pool(name="lscr", bufs=2) as pool_l,
        tc.tile_pool(name="xt_ps", bufs=3, space="PSUM") as pool_xtps,
        tc.tile_pool(name="eps", bufs=1, space="PSUM") as pool_eps,
        tc.tile_pool(name="xtsb", bufs=3) as pool_xtsb,
    ):
        m_prev = None
        for c in range(NCH):
            i0 = c * CH
            e_sb = pool_esb.tile([128, NG, CH, K], F32, tag="esb")
            m_c = pool_m.tile([128, CH, NG, K], F32, tag="mout")
            for g in range(NG):
                # emissions for (group g, positions i0..i0+CH)
                for h in range(CH // EPS):
                    eps = pool_eps.tile([128, EPS * K], F32, tag=f"eps{g}")
                    for ii in range(EPS):
                        pos = h * EPS + ii
                        sc, si = divmod(pos, XCH)
                        if si == 0:
                            xs = pool_x.tile([128, XCH * D], F32, tag=f"xs{g}")
                            nc.sync.dma_start(
                                xs[:],
                                X[g * 128 : (g + 1) * 128, i0 + sc * XCH : i0 + (sc + 1) * XCH, :],
                            )
                            xs3 = xs[:].rearrange("p (i d) -> p i d", d=D)
                        xt_ps = pool_xtps.tile([128, 128], F32, tag="xt")
                        nc.tensor.transpose(xt_ps[:], xs3[:, si, :], ident[:])
                        xt_sb = pool_xtsb.tile([128, 128], F32, tag="xt_sb")
                        nc.scalar.copy(xt_sb[:], xt_ps[:])
                        nc.tensor.matmul(
                            eps[:, ii * K : (ii + 1) * K],
                            lhsT=xt_sb[:],
                            rhs=wt[:],
                            start=True,
                            stop=True,
                            skip_group_check=True,
                        )
                    nc.scalar.copy(
                        e_sb[:, g, h * EPS : (h + 1) * EPS, :],
                        eps[:].rearrange("p (i k) -> p i k", k=K),
                    )
            for g in range(NG):
                nc.sync.dma_start(
                    e_dram[i0 : i0 + CH, g * 128 : (g + 1) * 128, :].rearrange(
                        "i p k -> p i k"
                    ),
                    e_sb[:, g, :, :],
                )
            # forward scan over this chunk
            for pos in range(CH):
                i = i0 + pos
                if i == 0:
                    nc.vector.tensor_copy(m_c[:, 0, :, :], e_sb[:, :, 0, :])
                    continue
                mp = m_prev[:, CH - 1, :, :] if pos == 0 else m_c[:, pos - 1, :, :]
                if use_fused == "fuse4":
                    s_all = pool_s.tile([128, NG, K, K], F32, tag="sall")
                    _custom_dve_raw(
                        nc.vector,
                        segmax,
                        out=s_all[:],
                        in0=mp.unsqueeze(2).broadcast_to([128, NG, K, K]),
                        in1=trepf3.unsqueeze(1).broadcast_to([128, NG, K, K]),
                    )
                    lpart = s_all[:, :, :, K - 1]
                elif use_fused:
                    s_all = pool_s.tile([128, NG, K, K], F32, tag="sall")
                    for g in range(NG):
                        nc.vector._custom_dve(
                            segmax,
                            out=s_all[:, g, :, :],
                            in0=mp[:, g, :].unsqueeze(1).broadcast_to([128, K, K]),
                            in1=trepf3,
                        )
                    lpart = s_all[:, :, :, K - 1]
                else:
                    s_all = pool_s.tile([128, NG, K, K], F32, tag="sall")
                    nc.vector.tensor_tensor(
                        s_all[:],
                        mp.unsqueeze(2).broadcast_to([128, NG, K, K]),
                        trepf3.unsqueeze(1).broadcast_to([128, NG, K, K]),
                        mybir.AluOpType.add,
                    )
                    l_scr = pool_l.tile([128, NG, K], F32, tag="lscr")
                    nc.vector.tensor_reduce(
                        l_scr[:], s_all[:], mybir.AxisListType.X, mybir.AluOpType.max
                    )
                    lpart = l_scr[:]
                adds.tensor_tensor(
                    m_c[:, pos, :, :], e_sb[:, :, pos, :], lpart, mybir.AluOpType.add
                )
            for g in range(NG):
                nc.sync.dma_start(
                    m_dram[i0 : i0 + CH, g * 128 : (g + 1) * 128, :].rearrange(
                        "i p k -> p i k"
                    ),
                    m_c[:, :, g, :],
                )
            m_prev = m_c

    # ---------------- Phase B: backward scan + labels ----------------
    with (
        tc.tile_pool(name="ein", bufs=3) as pool_ein,
        tc.tile_pool(name="min", bufs=3) as pool_min,
        tc.tile_pool(name="tch", bufs=2) as pool_t,
        tc.tile_pool(name="s2", bufs=3) as pool_s2,
        tc.tile_pool(name="gpp", bufs=3) as pool_g,
        tc.tile_pool(name="bulk", bufs=2) as pool_bulk,
    ):
        G = None
        for c in reversed(range(NCH)):
            i0 = c * CH
            e_in = pool_ein.tile([128, CH, NG, K], F32, tag="ein")
            m_in = pool_min.tile([128, CH, NG, K], F32, tag="min")
            for g in range(NG):
                nc.sync.dma_start(
                    e_in[:, :, g, :],
                    e_dram[i0 : i0 + CH, g * 128 : (g + 1) * 128, :].rearrange(
                        "i p k -> p i k"
                    ),
                )
                nc.sync.dma_start(
                    m_in[:, :, g, :],
                    m_dram[i0 : i0 + CH, g * 128 : (g + 1) * 128, :].rearrange(
                        "i p k -> p i k"
                    ),
                )
            t_c = pool_t.tile([128, CH, NG, K], F32, tag="tch")
            for pos in reversed(range(CH)):
                i = i0 + pos
                if i == L - 1:
                    G = pool_g.tile([128, NG, K], F32, tag="g")
                    nc.vector.tensor_copy(G[:], e_in[:, pos, :, :])
                    nc.vector.tensor_copy(t_c[:, pos, :, :], m_in[:, pos, :, :])
                    continue
                s2 = pool_s2.tile([128, NG, K, K], F32, tag="s2")
                if use_fused == "fuse4":
                    _custom_dve_raw(
                        nc.vector,
                        segmax,
                        out=s2[:],
                        in0=G[:].unsqueeze(2).broadcast_to([128, NG, K, K]),
                        in1=trepg3.unsqueeze(1).broadcast_to([128, NG, K, K]),
                    )
                    maxpart = s2[:, :, :, K - 1]
                elif use_fused:
                    for g in range(NG):
                        nc.vector._custom_dve(
                            segmax,
                            out=s2[:, g, :, :],
                            in0=G[:, g, :].unsqueeze(1).broadcast_to([128, K, K]),
                            in1=trepg3,
                        )
                    maxpart = s2[:, :, :, K - 1]
                else:
                    nc.vector.tensor_tensor(
                        s2[:],
                        G[:].unsqueeze(2).broadcast_to([128, NG, K, K]),
                        trepg3.unsqueeze(1).broadcast_to([128, NG, K, K]),
                        mybir.AluOpType.add,
                    )
                    mp_scr = pool_g.tile([128, NG, K], F32, tag="mpart")
                    nc.vector.tensor_reduce(
                        mp_scr[:], s2[:], mybir.AxisListType.X, mybir.AluOpType.max
                    )
                    maxpart = mp_scr[:]
                Gn = pool_g.tile([128, NG, K], F32, tag="g")
                gadd_eng = nc.gpsimd if adds_engine == "gadd_gpsimd" else adds
                gadd_eng.tensor_tensor(
                    Gn[:], e_in[:, pos, :, :], maxpart, mybir.AluOpType.add
                )
                # t feeds only the end-of-chunk bulk argmax (latency-tolerant)
                # -> run it on the otherwise-idle GPSIMD to shorten the DVE
                # stream, which real HW shows is per-op-overhead sensitive.
                nc.gpsimd.tensor_tensor(
                    t_c[:, pos, :, :], m_in[:, pos, :, :], maxpart, mybir.AluOpType.add
                )
                G = Gn
            # bulk argmax over y for this chunk
            tmax = pool_bulk.tile([128, CH, NG], F32, tag="tmax")
            nc.vector.tensor_reduce(
                tmax[:], t_c[:], mybir.AxisListType.X, mybir.AluOpType.max
            )
            mask = pool_bulk.tile([128, CH, NG, K], F32, tag="mask")
            nc.vector.tensor_tensor(
                mask[:],
                t_c[:],
                tmax[:].unsqueeze(3).broadcast_to([128, CH, NG, K]),
                mybir.AluOpType.is_equal,
            )
            cand = pool_bulk.tile([128, CH, NG, K], F32, tag="mask")
            nc.vector.tensor_tensor(
                cand[:],
                mask[:],
                revy[:].unsqueeze(1).unsqueeze(1).broadcast_to([128, CH, NG, K]),
                mybir.AluOpType.mult,
            )
            rc = pool_bulk.tile([128, CH, NG], F32, tag="tmax")
            nc.vector.tensor_reduce(
                rc[:], cand[:], mybir.AxisListType.X, mybir.AluOpType.max
            )
            lblf = pool_bulk.tile([128, CH, NG], F32, tag="lblf")
            nc.vector.tensor_scalar(
                lblf[:], rc[:], -1.0, 26.0, mybir.AluOpType.mult, mybir.AluOpType.add
            )
            nc.vector.tensor_copy(
                labels_sb[:, :, i0 : i0 + CH].transpose([0, 2, 1]), lblf[:]
            )
    for g in range(NG):
        nc.sync.dma_start(labels_out[g * 128 : (g + 1) * 128, :], labels_sb[:, g, :])


# ---------------------------------------------------------------------------
# Host-side driver
# ---------------------------------------------------------------------------


def _host_consts(W, T):
    K_, D_ = W.shape
    assert (K_, D_) == (K, D)
    wt = np.ascontiguousarray(W.T).astype(np.float32)  # [128, 26]
    trepf = np.tile(np.ascontiguousarray(T.T).reshape(1, -1), (128, 1)).astype(np.float32)
    trepg = np.tile(np.ascontiguousarray(T).reshape(1, -1), (128, 1)).astype(np.float32)
    ident = np.eye(128, dtype=np.float32)
    revy = np.tile((26.0 - np.arange(K, dtype=np.float32))[None], (128, 1))
    return {"wt": wt, "trepf": trepf, "trepg": trepg, "ident": ident, "revy": revy}


_prog_cache = {}


def build_program(L=512, use_fused=True, adds_engine="vector"):
    key = (L, use_fused, adds_engine)
    if key in _prog_cache:
        return _prog_cache[key]
    from contextlib import ExitStack

    nc = bacc.Bacc("TRN2", target_bir_lowering=False, debug=False)
    in_aps = {
        "X": nc.dram_tensor("X", [NG * 128, L, D], F32, kind="ExternalInput").ap(),
        "wt": nc.dram_tensor("wt", [128, K], F32, kind="ExternalInput").ap(),
        "trepf": nc.dram_tensor("trepf", [128, K * K], F32, kind="ExternalInput").ap(),
        "trepg": nc.dram_tensor("trepg", [128, K * K], F32, kind="ExternalInput").ap(),
        "ident": nc.dram_tensor("ident", [128, 128], F32, kind="ExternalInput").ap(),
        "revy": nc.dram_tensor("revy", [128, K], F32, kind="ExternalInput").ap(),
    }
    out_aps = {
        "labels": nc.dram_tensor("labels", [NG * 128, L], I32, kind="ExternalOutput").ap()
    }
    with tile.TileContext(nc) as tc:
        with ExitStack() as ctx:
            build_crf_kernel(
                ctx, tc, out_aps, in_aps, L=L, use_fused=use_fused, adds_engine=adds_engine
            )
    nc.compile()
    _prog_cache[key] = nc
    return nc


def kernel(X, W, T):
    X = np.ascontiguousarray(X, dtype=np.float32)
    W = np.ascontiguousarray(W, dtype=np.float32)
    T = np.ascontiguousarray(T, dtype=np.float32)
    B, L, D_ = X.shape
    wpc = B // NUM_CORES
    assert wpc == NG * 128 and D_ == D

    consts = _host_consts(W, T)
    nc = build_program(L=L, use_fused=True)
    in_maps = []
    for c in range(NUM_CORES):
        m = {"X": X[c * wpc : (c + 1) * wpc]}
        m.update(consts)
        in_maps.append(m)
    res = run_bass_kernel_spmd(nc, in_maps, list(range(NUM_CORES)))
    out = np.concatenate([r["labels"] for r in res.results], axis=0)
    return out.astype(np.int32)


if __name__ == "__main__":
    # smoke test at small L against a numpy reference
    rng = np.random.default_rng(0)
    L = 64
    X = rng.standard_normal((NUM_CORES * NG * 128, L, D)).astype(np.float32)
    W = rng.standard_normal((K, D)).astype(np.float32)
    T = rng.standard_normal((K, K)).astype(np.float32)
    lab = kernel(X, W, T)
    print(lab.shape, lab.dtype, lab[:2, :8])



# revision 2
# speedup vs baseline: 17.3373x; 17.3373x over previous
"""CRF Viterbi decode kernel v2 for Trainium2 (8 NeuronCores, data parallel).

Problem: X [4096, 512, 128] f32, W [26, 128], T [26, 26]
  e = einsum('bld,kd->blk', X, W); per-word Viterbi max-sum over L=512,
  labels [4096, 512] int32 via forward (m) + backward (G/maxpart) scans:
      m[i,y]  = e[i,y] + max_j(m[i-1,j] + T[j,y]),   m[0] = e[0]
      mp[i,j] = max_y(T[j,y] + G[i+1,y]),  G[i] = e[i] + mp[i], G[L-1]=e[L-1]
      label[i] = argmax_y(m[i,y] + mp[i,y])   (mp[L-1] := 0)

v2 structure (vs v1):
  - X is transposed HOST-side to [L, D, W] per core, so emission matmuls
    load lhsT directly from DRAM: no PE transposes, no PSUM->SBUF xt
    copies (saves ~0.5ms of PE/ACT work and the PE-sequencer ldweights
    churn of 2048 per-tile weight loads).
  - e is NOT spilled to DRAM: phase B recomputes emissions on the
    otherwise-idle PE/ACT (cuts DRAM scratch traffic by 3/4; the m spill
    uses the SBUF-native [g][p][i][k] layout: 128 descriptors per DMA
    instead of 4096, which removed ~1M DMA descriptors vs v1).
  - phase-B t-adds ride the Pool engine (ADD is one of the few ops the
    Q7 software gpsimd implements; MAX/compares are DVE-only on real HW,
    so all max-plus scan work stays on DVE's custom SEG_MAX_ADD op).
  - m spills are deferred one chunk and all DMA queues are split by
    dependency (X2 loads on SP, spills/pool-loads on ACT) so no in-order
    queue ever head-of-line blocks a prefetch.
  - ndve=3 duty-cycled Pool-chain variants exist but are dead code on
    real HW (Pool lacks MAX); ndve=4 is the shipping config.
"""

import os
import sys

for _p in ("/opt/trn_rl_repo", "/root/.axon_site/_ro/trn_rl_repo"):
    if os.path.isdir(_p) and _p not in sys.path:
        sys.path.append(_p)

import dataclasses
import numpy as np

import concourse.bass as bass
import concourse.tile as tile
from concourse import bacc, mybir
from concourse.bass_utils import run_bass_kernel_spmd

F32 = mybir.dt.float32
I32 = mybir.dt.int32

NUM_CORES = 8
K = 26
D = 128
NG = 4  # word groups of 128 per core
W_PER_CORE = NG * 128
CH_CONST = 32  # scan/spill chunk (positions); rmask const sized to CH*K
# Bias making all Pool-chain scores positive (max |score| ~ 11.7k for this
# input distribution), so the multiplicative-reset segmented max scan works.
CSHIFT = 16384.0

# ---------------------------------------------------------------------------
# Custom DVE op: segmented (per-page) running max of (Src0 + Src1).
# (identical to v1's op)
# ---------------------------------------------------------------------------

_SEGMAX_NAME = "SEG_MAX_ADD_ANT"
_segmax_op = None


def _segmax_reference(in0, in1, c0, c1, c2):
    z = np.asarray(in0, np.float32) + np.asarray(in1, np.float32)
    return np.maximum.accumulate(z, axis=-1)


def _build_segmax_uops(ver):
    from concourse import dve_spec as ds
    from concourse.dve_spec import AluOp, Spec, Src0, Src1, Trigger, scan

    spec = Spec(body=scan(AluOp.MAX, Src0 + Src1), reference=_segmax_reference)
    ds._validate_body(spec, ver)
    spec2 = ds._hoist_stream_invariant_ops(spec)
    scans = ds._collect(spec2.body, ds.Scan)
    placement = ds._build_placement(spec2, scans, ds.N_STAGES[ver], ds.N_LANES[ver])
    base = ds._build_state_machine(spec2, scans, [], placement)
    assert len(base) == 2  # [seed, steady]
    d = placement.node_stage[scans[0]]
    steady_stage = placement.pipeline[d]
    seed = base[0]
    steady = dataclasses.replace(
        base[1],
        trigger=(Trigger.SRC_TENSOR_DONE, Trigger.SUB_DIM_DONE, Trigger.NONE),
        next=(0, 2, 0),
    )
    step = dataclasses.replace(
        base[1],
        trigger=(Trigger.SRC_TENSOR_DONE, Trigger.SUB_DIM_DONE, Trigger.COUNT),
        next=(0, 2, 1),
        repeat=1,
        overrides={d: ds._Stage(AluOp.MAX, ds.MaxNeg, steady_stage.b)},
    )
    uops = [ds._assemble(s) for s in (seed, steady, step)]
    for u in uops:
        u.validate(ver)
    return spec, uops


def get_segmax_op():
    global _segmax_op
    if _segmax_op is not None:
        return _segmax_op
    from concourse import dve_ops
    from concourse.dve_ops import OPS, CUSTOM_DVE_SPECS, _SUB_OPCODE_FOR_NAME, DveOp
    from concourse.dve_uop import DveOpSpec

    @dataclasses.dataclass(frozen=True)
    class _HandBuiltDveOp(DveOp):
        def compile(self, ver):
            key = (self.name, ver)
            if (r := dve_ops._COMPILE_CACHE.get(key)) is not None:
                return r
            from concourse.dve_ops import get_dve_sub_opcode

            _, uops = _build_segmax_uops(ver)
            result = DveOpSpec(
                name=self.name,
                opcode=get_dve_sub_opcode(self.name),
                uops=uops,
                rd1_en=True,
            )
            dve_ops._COMPILE_CACHE[key] = result
            return result

    spec, _ = _build_segmax_uops("v3")
    op = _HandBuiltDveOp(_SEGMAX_NAME, spec, subdim=True, uops_sha={})
    if _SEGMAX_NAME not in _SUB_OPCODE_FOR_NAME:
        OPS.append(op)
        CUSTOM_DVE_SPECS[_SEGMAX_NAME] = spec
        _SUB_OPCODE_FOR_NAME[_SEGMAX_NAME] = 1 + len(OPS) - 1
        assert _SUB_OPCODE_FOR_NAME[_SEGMAX_NAME] < 0x20
    _segmax_op = op
    return op


def _custom_dve_raw(vec, op, out, in0, in1):
    """One InstCustomDveAnt over 3-free-dim APs (multi-chain fused)."""
    import concourse.bass_isa as bass_isa
    from concourse.dve_ops import get_dve_sub_opcode
    from concourse.dve_table_gen import dve_ver_for
    from concourse import mybir as mb

    nc = vec.bass
    if op.name not in nc.m.ant_custom_dve_ops:
        nc.m.ant_custom_dve_ops = sorted({*nc.m.ant_custom_dve_ops, op.name})
    compiled = op.compile(dve_ver_for(nc.trn_type))
    shape = bass_isa.CustomDveShape.STT
    isa_opcode = nc.isa.Opcode[
        f"NEURON_ISA_TPB_OPCODE_CUSTOM_DVE_ANT_{shape.slot()}"
    ].value
    imm = mb.ImmediateValue(dtype=mb.dt.float32, value=0.0)
    return vec.add_instruction(
        bass_isa.InstCustomDveAnt(
            name=nc.get_next_instruction_name(),
            op_name=op.name,
            rd1_en=compiled.rd1_en,
            subdim=0x02 if op.subdim else 0,
            imm2=0.0,
            shape=shape,
            row=get_dve_sub_opcode(op.name),
            isa_opcode=isa_opcode,
            ins=[
                vec.lower_ap(in0, for_isa=True, opt=False),
                vec.lower_ap(in1, for_isa=True, opt=False),
                imm,
                mb.ImmediateValue(dtype=mb.dt.float32, value=0.0),
            ],
            outs=[vec.lower_ap(out, for_isa=True, opt=False)],
        )
    )


# ---------------------------------------------------------------------------
# Program builder
# ---------------------------------------------------------------------------


def build_crf_kernel(ctx, tc, out_aps, in_aps, L=512, ndve=3, patA="PPPD", patB="PPPD"):
    """Emit the per-core CRF decode program.

    in_aps DRAM: X2 [L, D, W_PER_CORE] (host-transposed X), wt [128, K],
      trepf [128, K*K] (T^T flat: [y*26+j] = T[j,y]),
      trepg [128, K*K] (T flat:  [j*26+y] = T[j,y]),
      revy [128, K] (= 26 - y), rmask (unused, kept for input compat).
    out_aps: labels [W_PER_CORE, L] int32.

    ndve: 4 -> all chains on DVE; 3 -> chain 3 duty-cycled between Pool
    (TensorTensor add + overlap-halving max tree) and DVE (segmax) at CHUNK
    granularity per patA/patB ('P'/'D' per chunk), running one chunk behind
    chains 0-2 so neither engine's in-order queue ever waits on the other.
    """
    nc = tc.nc
    X2 = in_aps["X2"]
    labels_out = out_aps["labels"]

    KK = K * K
    CH = CH_CONST  # scan/spill chunk (positions)
    assert L % CH == 0
    NCH = L // CH
    XCH = 4  # X staging positions per tile
    EPS = 16  # emission positions per PSUM tile (16*26=416 f32 <= 512)
    npool = NG - ndve
    assert 0 <= npool <= 1, "at most one Pool chain supported"

    segmax = get_segmax_op()

    AX = mybir.AxisListType.X
    ADD = mybir.AluOpType.add
    MAX = mybir.AluOpType.max
    ISEQ = mybir.AluOpType.is_equal
    MULT = mybir.AluOpType.mult

    def pool_tree_lastdim(ap, width):
        """In-place overlap-halving max over the last dim on Pool; result in
        ap[..., 0]. Head-writing keeps in-place reads ahead of writes in
        stream order."""
        w = width
        while w > 1:
            h = (w + 1) // 2
            nc.gpsimd.tensor_tensor(ap[..., 0:h], ap[..., 0:h], ap[..., w - h : w], MAX)
            w = h

    # DRAM scratch, SBUF-native layout: [g][p][i][k]
    m_dram = nc.dram_tensor(f"m_scr_{L}", [NG, 128, L, K], F32).ap()

    cpool = ctx.enter_context(tc.tile_pool(name="consts", bufs=1))
    trepf = cpool.tile([128, KK], F32, tag="trepf")
    trepg = cpool.tile([128, KK], F32, tag="trepg")
    wt = cpool.tile([128, K], F32, tag="wt")
    revy = cpool.tile([128, K], F32, tag="revy")
    nc.sync.dma_start(trepf[:], in_aps["trepf"])
    nc.sync.dma_start(trepg[:], in_aps["trepg"])
    nc.sync.dma_start(wt[:], in_aps["wt"])
    nc.sync.dma_start(revy[:], in_aps["revy"])
    trepf3 = trepf[:].rearrange("p (y j) -> p y j", j=K)
    trepg3 = trepg[:].rearrange("p (j y) -> p j y", y=K)

    labels_pool = ctx.enter_context(tc.tile_pool(name="labels", bufs=1))
    labels_sb = labels_pool.tile([128, NG, L], I32)

    def emit_chunk_emissions(i0, e_d, e_p, pool_x, pool_eps):
        """Emissions for positions [i0, i0+CH): X2 loads -> matmuls -> ACT
        copies into e_d [128, ndve, CH, K] / e_p [128, CH, K]."""
        for h in range(CH // EPS):
            eps_t = [
                pool_eps.tile([128, EPS * K], F32, tag=f"eps{g}", name=f"eps{g}")
                for g in range(NG)
            ]
            xs = None
            for ii in range(EPS):
                pos = h * EPS + ii
                sc, si = divmod(pos, XCH)
                if si == 0:
                    xs = pool_x.tile([128, XCH, NG * 128], F32, tag="xs", name="xs")
                    nc.sync.dma_start(
                        xs[:],
                        X2[i0 + sc * XCH : i0 + (sc + 1) * XCH, :, :].rearrange(
                            "i d w -> d i w"
                        ),
                    )
                for g in range(NG):
                    nc.tensor.matmul(
                        eps_t[g][:, ii * K : (ii + 1) * K],
                        lhsT=xs[:, si, g * 128 : (g + 1) * 128],
                        rhs=wt[:],
                        start=True,
                        stop=True,
                        skip_group_check=True,
                    )
            for g in range(NG):
                srcv = eps_t[g][:].rearrange("p (i k) -> p i k", k=K)
                if g < ndve:
                    nc.scalar.copy(e_d[:, g, h * EPS : (h + 1) * EPS, :], srcv)
                else:
                    nc.scalar.copy(e_p[:, h * EPS : (h + 1) * EPS, :], srcv)

    # ---------------- Phase A: emissions + forward scan ----------------
    with (
        tc.tile_pool(name="xstage", bufs=3) as pool_x,
        tc.tile_pool(name="ed", bufs=3) as pool_ed,
        tc.tile_pool(name="ep", bufs=4) as pool_ep,
        tc.tile_pool(name="md", bufs=2) as pool_md,
        tc.tile_pool(name="mp", bufs=3) as pool_mp,
        tc.tile_pool(name="sd", bufs=2) as pool_sd,
        tc.tile_pool(name="sp", bufs=2) as pool_sp,
        tc.tile_pool(name="eps", bufs=2, space="PSUM") as pool_eps,
    ):
        md_prev = None
        pending_spills = []
        pending_g3 = None  # deferred chain-3 forward chunk closure

        def g3_forward_chunk(c, e_p, m_p, mp_prev):
            i0 = c * CH
            on_dve = patA[c % len(patA)] == "D"
            for pos in range(CH):
                i = i0 + pos
                if i == 0:
                    eng = nc.vector if on_dve else nc.gpsimd
                    eng.tensor_copy(m_p[:, 0, :], e_p[:, 0, :])
                    continue
                mpp = m_p[:, pos - 1, :] if pos else mp_prev[:, CH - 1, :]
                s_p = pool_sp.tile([128, K, K], F32, tag="sp", name="s_p")
                if on_dve:
                    nc.vector._custom_dve(
                        segmax,
                        out=s_p[:],
                        in0=mpp.unsqueeze(1).broadcast_to([128, K, K]),
                        in1=trepf3,
                    )
                    nc.vector.tensor_tensor(
                        m_p[:, pos, :], e_p[:, pos, :], s_p[:, :, K - 1], ADD
                    )
                else:
                    nc.gpsimd.tensor_tensor(
                        s_p[:],
                        trepf3,
                        mpp.unsqueeze(1).broadcast_to([128, K, K]),
                        ADD,
                    )
                    pool_tree_lastdim(s_p[:], K)
                    nc.gpsimd.tensor_tensor(
                        m_p[:, pos, :], e_p[:, pos, :], s_p[:, :, 0], ADD
                    )
            pending_spills.append((m_dram[NG - 1, :, i0 : i0 + CH, :], m_p[:]))

        mp_hist = {}
        for c in range(NCH):
            i0 = c * CH
            e_d = pool_ed.tile([128, ndve, CH, K], F32, tag="ed")
            e_p = (
                pool_ep.tile([128, CH, K], F32, tag="ep", name="e_p") if npool else None
            )
            m_d = pool_md.tile([128, CH, ndve, K], F32, tag="md")
            m_p = (
                pool_mp.tile([128, CH, K], F32, tag="mp", name="m_p") if npool else None
            )
            mp_hist[c] = m_p

            emit_chunk_emissions(i0, e_d, e_p, pool_x, pool_eps)

            # deferred m spills (see below) after this chunk's emission copies
            for dst, srcv in pending_spills:
                nc.scalar.dma_start(dst, srcv)
            pending_spills = []

            # forward scan over this chunk, chains 0..ndve-1
            for pos in range(CH):
                i = i0 + pos
                if i == 0:
                    nc.vector.tensor_copy(m_d[:, 0, :, :], e_d[:, :, 0, :])
                    continue
                mdp = (
                    md_prev[:, CH - 1, :, :] if pos == 0 else m_d[:, pos - 1, :, :]
                )
                s_d = pool_sd.tile([128, ndve, K, K], F32, tag="sd")
                for g in range(ndve):
                    nc.vector._custom_dve(
                        segmax,
                        out=s_d[:, g, :, :],
                        in0=mdp[:, g, :].unsqueeze(1).broadcast_to([128, K, K]),
                        in1=trepf3,
                    )
                nc.vector.tensor_tensor(
                    m_d[:, pos, :, :],
                    e_d[:, :, pos, :],
                    s_d[:, :, :, K - 1],
                    ADD,
                )

            # chain 3 runs one chunk behind (deferred closure)
            if npool:
                if pending_g3 is not None:
                    pending_g3()
                pc = c
                pending_g3 = (
                    lambda pc=pc, e_p=e_p, m_p=m_p: g3_forward_chunk(
                        pc, e_p, m_p, mp_hist.get(pc - 1)
                    )
                )

            for g in range(ndve):
                pending_spills.append(
                    (m_dram[g, :, i0 : i0 + CH, :], m_d[:, :, g, :])
                )
            md_prev = m_d
        if pending_g3 is not None:
            pending_g3()
        for dst, srcv in pending_spills:
            nc.scalar.dma_start(dst, srcv)
        pending_spills = []

    # ------- Phase B: emissions (recomputed) + backward scan + labels -------
    with (
        tc.tile_pool(name="xstageb", bufs=2) as pool_xb,
        tc.tile_pool(name="edb", bufs=2) as pool_edb,
        tc.tile_pool(name="epb", bufs=3) as pool_epb,
        tc.tile_pool(name="mind", bufs=3) as pool_mind,
        tc.tile_pool(name="minp", bufs=3) as pool_minp,
        tc.tile_pool(name="s2d", bufs=2) as pool_s2d,
        tc.tile_pool(name="s2p", bufs=2) as pool_s2p,
        tc.tile_pool(name="gd", bufs=3) as pool_gd,
        tc.tile_pool(name="gp", bufs=3) as pool_gp,
        tc.tile_pool(name="td", bufs=2) as pool_td,
        tc.tile_pool(name="tp", bufs=3) as pool_tp,
        tc.tile_pool(name="blkd", bufs=2) as pool_blkd,
        tc.tile_pool(name="blkp", bufs=2) as pool_blkp,
        tc.tile_pool(name="epsb", bufs=2, space="PSUM") as pool_epsb,
    ):
        G_d = None
        G_state = [None]  # chain-3 backward state across deferred chunks
        pending_g3b = None

        def g3_backward_chunk(c, e_inp, m_inp, t_p):
            i0 = c * CH
            on_dve = patB[c % len(patB)] == "D"
            for pos in reversed(range(CH)):
                i = i0 + pos
                if i == L - 1:
                    eng = nc.vector if on_dve else nc.gpsimd
                    G_p = pool_gp.tile([128, K], F32, tag="gp", name="G_p")
                    eng.tensor_copy(G_p[:], e_inp[:, pos, :])
                    eng.tensor_copy(t_p[:, pos, :], m_inp[:, pos, :])
                    G_state[0] = G_p
                    continue
                G_p = G_state[0]
                s2_p = pool_s2p.tile([128, K, K], F32, tag="s2p", name="s2_p")
                if on_dve:
                    nc.vector._custom_dve(
                        segmax,
                        out=s2_p[:],
                        in0=G_p[:].unsqueeze(1).broadcast_to([128, K, K]),
                        in1=trepg3,
                    )
                    mpart_p = s2_p[:, :, K - 1]
                    Gn_p = pool_gp.tile([128, K], F32, tag="gp", name="Gn_p")
                    nc.vector.tensor_tensor(Gn_p[:], e_inp[:, pos, :], mpart_p, ADD)
                    nc.vector.tensor_tensor(
                        t_p[:, pos, :], m_inp[:, pos, :], mpart_p, ADD
                    )
                else:
                    nc.gpsimd.tensor_tensor(
                        s2_p[:],
                        trepg3,
                        G_p[:].unsqueeze(1).broadcast_to([128, K, K]),
                        ADD,
                    )
                    pool_tree_lastdim(s2_p[:], K)
                    mpart_p = s2_p[:, :, 0]
                    Gn_p = pool_gp.tile([128, K], F32, tag="gp", name="Gn_p")
                    nc.gpsimd.tensor_tensor(Gn_p[:], e_inp[:, pos, :], mpart_p, ADD)
                    nc.gpsimd.tensor_tensor(
                        t_p[:, pos, :], m_inp[:, pos, :], mpart_p, ADD
                    )
                G_state[0] = Gn_p
            # group-3 labels for this chunk (Pool trees + DVE scale/copy)
            tmt_p = pool_blkp.tile([128, CH, K], F32, tag="tmtp")
            nc.gpsimd.tensor_copy(tmt_p[:], t_p[:])
            pool_tree_lastdim(tmt_p[:], K)
            mask_p = pool_blkp.tile([128, CH, K], F32, tag="maskp")
            nc.gpsimd.tensor_tensor(
                mask_p[:],
                t_p[:],
                tmt_p[:, :, 0:1].broadcast_to([128, CH, K]),
                ISEQ,
            )
            cand_p = pool_blkp.tile([128, CH, K], F32, tag="maskp")
            nc.gpsimd.tensor_tensor(
                cand_p[:],
                mask_p[:],
                revy[:].unsqueeze(1).broadcast_to([128, CH, K]),
                MULT,
            )
            pool_tree_lastdim(cand_p[:], K)
            lbl_p = pool_blkp.tile([128, CH], F32, tag="lblp")
            nc.vector.tensor_scalar(lbl_p[:], cand_p[:, :, 0], -1.0, 26.0, MULT, ADD)
            nc.vector.tensor_copy(labels_sb[:, NG - 1, i0 : i0 + CH], lbl_p[:])

        for c in reversed(range(NCH)):
            i0 = c * CH
            e_ind = pool_edb.tile([128, ndve, CH, K], F32, tag="edb", name="e_ind")
            e_inp = (
                pool_epb.tile([128, CH, K], F32, tag="epb", name="e_inp")
                if npool
                else None
            )
            emit_chunk_emissions(i0, e_ind, e_inp, pool_xb, pool_epsb)

            m_ind = pool_mind.tile([128, CH, ndve, K], F32, tag="mind")
            for g in range(ndve):
                nc.sync.dma_start(m_ind[:, :, g, :], m_dram[g, :, i0 : i0 + CH, :])
            if npool:
                m_inp = pool_minp.tile([128, CH, K], F32, tag="minp", name="m_inp")
                nc.scalar.dma_start(m_inp[:], m_dram[NG - 1, :, i0 : i0 + CH, :])
            t_d = pool_td.tile([128, CH, ndve, K], F32, tag="td")

            for pos in reversed(range(CH)):
                i = i0 + pos
                if i == L - 1:
                    G_d = pool_gd.tile([128, ndve, K], F32, tag="gd")
                    nc.vector.tensor_copy(G_d[:], e_ind[:, :, pos, :])
                    nc.vector.tensor_copy(t_d[:, pos, :, :], m_ind[:, pos, :, :])
                    continue
                s2_d = pool_s2d.tile([128, ndve, K, K], F32, tag="s2d")
                for g in range(ndve):
                    nc.vector._custom_dve(
                        segmax,
                        out=s2_d[:, g, :, :],
                        in0=G_d[:, g, :].unsqueeze(1).broadcast_to([128, K, K]),
                        in1=trepg3,
                    )
                mpart_d = s2_d[:, :, :, K - 1]
                Gn_d = pool_gd.tile([128, ndve, K], F32, tag="gd")
                nc.vector.tensor_tensor(Gn_d[:], e_ind[:, :, pos, :], mpart_d, ADD)
                # t is off the serial chain; Pool ADD is one of the few legal
                # Pool ops, so give it the t-adds.
                nc.gpsimd.tensor_tensor(
                    t_d[:, pos, :, :], m_ind[:, pos, :, :], mpart_d, ADD
                )
                G_d = Gn_d

            # chain 3 one chunk behind
            if npool:
                if pending_g3b is not None:
                    pending_g3b()
                t_p = pool_tp.tile([128, CH, K], F32, tag="tp", name="t_p")
                pending_g3b = (
                    lambda pc=c, e_inp=e_inp, m_inp=m_inp, t_p=t_p: g3_backward_chunk(
                        pc, e_inp, m_inp, t_p
                    )
                )

            # bulk argmax over y for this chunk, DVE chains
            tmax_d = pool_blkd.tile([128, CH, ndve], F32, tag="tmaxd")
            nc.vector.tensor_reduce(tmax_d[:], t_d[:], AX, MAX)
            mask_d = pool_blkd.tile([128, CH, ndve, K], F32, tag="maskd")
            nc.vector.tensor_tensor(
                mask_d[:],
                t_d[:],
                tmax_d[:].unsqueeze(3).broadcast_to([128, CH, ndve, K]),
                ISEQ,
            )
            cand_d = pool_blkd.tile([128, CH, ndve, K], F32, tag="maskd")
            nc.vector.tensor_tensor(
                cand_d[:],
                mask_d[:],
                revy[:].unsqueeze(1).unsqueeze(1).broadcast_to([128, CH, ndve, K]),
                MULT,
            )
            rc_d = pool_blkd.tile([128, CH, ndve], F32, tag="tmaxd")
            nc.vector.tensor_reduce(rc_d[:], cand_d[:], AX, MAX)
            lbl_d = pool_blkd.tile([128, CH, ndve], F32, tag="lbld")
            nc.vector.tensor_scalar(lbl_d[:], rc_d[:], -1.0, 26.0, MULT, ADD)
            nc.vector.tensor_copy(
                labels_sb[:, 0:ndve, i0 : i0 + CH].transpose([0, 2, 1]), lbl_d[:]
            )
        if pending_g3b is not None:
            pending_g3b()
    for g in range(NG):
        nc.sync.dma_start(labels_out[g * 128 : (g + 1) * 128, :], labels_sb[:, g, :])


# ---------------------------------------------------------------------------
# Host-side driver
# ---------------------------------------------------------------------------


def _host_consts(W, T):
    K_, D_ = W.shape
    assert (K_, D_) == (K, D)
    wt = np.ascontiguousarray(W.T).astype(np.float32)  # [128, 26]
    trepf = np.tile(np.ascontiguousarray(T.T).reshape(1, -1), (128, 1)).astype(
        np.float32
    )
    trepg = np.tile(np.ascontiguousarray(T).reshape(1, -1), (128, 1)).astype(np.float32)
    revy = np.tile((26.0 - np.arange(K, dtype=np.float32))[None], (128, 1))
    rm = np.ones(CH_CONST * K, dtype=np.float32)
    rm[::K] = 0.0  # reset at the first element of every K-wide page
    rmask = np.tile(rm[None], (128, 1))
    return {"wt": wt, "trepf": trepf, "trepg": trepg, "revy": revy, "rmask": rmask}


_prog_cache = {}


def build_program(L=512, ndve=4, patA="PPPD", patB="PPPD"):
    key = (L, ndve, patA, patB)
    if key in _prog_cache:
        return _prog_cache[key]
    from contextlib import ExitStack

    nc = bacc.Bacc("TRN2", target_bir_lowering=False, debug=False)
    in_aps = {
        "X2": nc.dram_tensor("X2", [L, D, W_PER_CORE], F32, kind="ExternalInput").ap(),
        "wt": nc.dram_tensor("wt", [128, K], F32, kind="ExternalInput").ap(),
        "trepf": nc.dram_tensor("trepf", [128, K * K], F32, kind="ExternalInput").ap(),
        "trepg": nc.dram_tensor("trepg", [128, K * K], F32, kind="ExternalInput").ap(),
        "revy": nc.dram_tensor("revy", [128, K], F32, kind="ExternalInput").ap(),
        "rmask": nc.dram_tensor(
            "rmask", [128, CH_CONST * K], F32, kind="ExternalInput"
        ).ap(),
    }
    out_aps = {
        "labels": nc.dram_tensor(
            "labels", [W_PER_CORE, L], I32, kind="ExternalOutput"
        ).ap()
    }
    with tile.TileContext(nc) as tc:
        with ExitStack() as ctx:
            build_crf_kernel(ctx, tc, out_aps, in_aps, L=L, ndve=ndve, patA=patA, patB=patB)
    nc.compile()
    _prog_cache[key] = nc
    return nc


def _transpose_X(X):
    """[B, L, D] -> per-core [L, D, W_PER_CORE] blocks, as one array
    [NUM_CORES, L, D, W_PER_CORE]."""
    B, L, D_ = X.shape
    nc_ = B // W_PER_CORE
    # [nc, W, L, D] -> [nc, L, D, W]
    Xr = X.reshape(nc_, W_PER_CORE, L, D_)
    return np.ascontiguousarray(np.transpose(Xr, (0, 2, 3, 1)))


def kernel(X, W, T):
    X = np.ascontiguousarray(X, dtype=np.float32)
    W = np.ascontiguousarray(W, dtype=np.float32)
    T = np.ascontiguousarray(T, dtype=np.float32)
    B, L, D_ = X.shape
    assert B % NUM_CORES == 0 and D_ == D
    assert B // NUM_CORES == W_PER_CORE

    consts = _host_consts(W, T)
    X2 = _transpose_X(X)
    nc = build_program(L=L)
    in_maps = []
    for c in range(NUM_CORES):
        m = {"X2": X2[c]}
        m.update(consts)
        in_maps.append(m)
    res = run_bass_kernel_spmd(nc, in_maps, list(range(NUM_CORES)))
    out = np.concatenate([r["labels"] for r in res.results], axis=0)
    return out.astype(np.int32)


if __name__ == "__main__":
    rng = np.random.default_rng(0)
    L = 64
    X = rng.standard_normal((NUM_CORES * W_PER_CORE, L, D)).astype(np.float32)
    W = rng.standard_normal((K, D)).astype(np.float32)
    T = rng.standard_normal((K, K)).astype(np.float32)
    lab = kernel(X, W, T)
    print(lab.shape, lab.dtype, lab[:2, :8])
